# revision 5
# baseline (speedup 1.0000x reference)
"""Trainium2 Bass kernel for CapsNet DigitCaps dynamic routing (3 iterations).

Reference math:
    u_hat[b,i,o,u] = sum_k W[i,o,u,k] * inp[b,i,k]   # [B,1152,10,16] = 189MB, never materialized here
    repeat 3x: c = softmax(b, o); s = sum_i c[i,o]*u_hat; v = squash(s);
               b += mean_b sum_u u_hat * v

Factored form used on device (u_hat eliminated -> everything stays in SBUF):
    s[b,ou]    = sum_{ik} inp[b,ik] * (W_t[ik,ou] * c[i,o])          # matmul over flattened (i,k)
    delta[i,o] = sum_{u,k} W_t[ik,ou] * (sum_b inp[b,ik]*v[b,ou])/B  # outer product + elementwise + reduce

Sharding: 8 cores, interleaved input-capsule shard i = core_id mod 8 (144 i's =
1152 (i,k) pairs per core = exactly 9 partition chunks of 128). inp/W are
pre-laid-out on the host. All routing iterations run in bf16 (1-pass matmuls,
fp32 PSUM accumulation). Cross-core traffic: bf16 AllReduce of the partial s
[256,160] (iterations 0/1) and a final fp32 ReduceScatter; each core then
squashes and emits its own 32-row batch slice of v, concatenated on the host.
b is core-local (each core owns its capsule rows; no collective needed for the
b update). squash's 1/sqrt is a fused-int magic seed + one Newton step on the
vector engine, so the only ACT table set ever loaded is exp's (no table
switching). The k-group reductions and c[i,o] -> (i,k)-partition broadcasts are
done on the tensor engine with small constant selection matrices (k8/sw); chunk
pairs share one PSUM bank so the vector engine consumes them in double-width
ops (start=True clears a whole PSUM bank, so each bank holds exactly one matmul
accumulation group). Collective bounce DMAs on the critical path use the
scalar-engine HWDGE ring.
"""

import numpy as np

N_CORES = 8
B = 256
IC, OC, OU, IK = 1152, 10, 16, 8
IL = IC // N_CORES          # 144 local input capsules
PK = IL * IK                # 1152 local (i,k) pairs
NCH = PK // 128             # 9 partition chunks
F = OC * OU                 # 160 free (o,u)
BT = B // 128               # 2 batch partition tiles
BSL = B // N_CORES          # 32-row batch slice per core (iteration 2)
MAGIC_I = 0x5F3759DF        # rsqrt seed integer

_CACHE = {}


def _build():
    import concourse.bacc as bacc
    import concourse.mybir as mybir
    import concourse.tile as tile

    fp32 = mybir.dt.float32
    bf16 = mybir.dt.bfloat16
    i32 = mybir.dt.int32
    AF = mybir.ActivationFunctionType
    ALU = mybir.AluOpType
    AX = mybir.AxisListType

    nc = bacc.Bacc("TRN2", target_bir_lowering=False, debug=False, num_devices=N_CORES)

    inp_bk_d = nc.dram_tensor("inp_bk", [128, BT, PK], bf16, kind="ExternalInput")
    inpT_d = nc.dram_tensor("inpT", [128, NCH, B], bf16, kind="ExternalInput")
    wt_d = nc.dram_tensor("wt", [128, NCH, F], bf16, kind="ExternalInput")
    k8_d = nc.dram_tensor("k8", [128, 4, 128], bf16, kind="ExternalInput")
    sw_d = nc.dram_tensor("sw", [128, 240], bf16, kind="ExternalInput")
    v_d = nc.dram_tensor("v_out", [BSL, F], fp32, kind="ExternalOutput")
    # Shared-space AllReduce outputs: lets ncfw use the one-shot
    # shared-output algorithm (HBM-HBM AR wants Shared for max performance)
    s_red_sh = [
        nc.dram_tensor(f"s_red{i}", [B, F], bf16, kind="Internal", addr_space="Shared")
        for i in range(2)
    ]

    with tile.TileContext(nc) as tc:
        with (
            tc.tile_pool(name="main", bufs=1) as mp,
            tc.tile_pool(name="ps", bufs=2, space="PSUM") as pp,
            tc.tile_pool(name="psb", bufs=1, space="PSUM") as ppb,
            tc.tile_pool(name="dram", bufs=2, space="DRAM") as dp,
        ):
            k8 = mp.tile([128, 4, 128], bf16, tag="k8", name="k8")
            nc.gpsimd.dma_start(k8[:], k8_d[:])

            scr = mp.tile([128, 64], fp32, tag="scr", name="scr")
            nc.vector.memset(scr[:], 1.0)

            inp_bk = mp.tile([128, BT, PK], bf16, tag="inp_bk", name="inp_bk")
            inpT = mp.tile([128, NCH, B], bf16, tag="inpT", name="inpT")
            wt = mp.tile([128, NCH, F], bf16, tag="wt", name="wt")
            sw = mp.tile([128, 240], bf16, tag="sw", name="sw")
            nc.gpsimd.dma_start(inp_bk[:], inp_bk_d[:])
            nc.gpsimd.dma_start(inpT[:], inpT_d[:])
            nc.gpsimd.dma_start(wt[:], wt_d[:])
            nc.gpsimd.dma_start(sw[:], sw_d[:])

            b_all = mp.tile([128, 2, OC], fp32, tag="b_all", name="b_all")
            nc.vector.memset(b_all[:], 0.0)

            # PE HAM warm chain: ACT<->PE ping-pong paces dummy matmuls in time
            wact = mp.tile([128, 16], bf16, tag="wact", name="wact")
            nc.vector.memset(wact[:], 1.0)
            wseed = mp.tile([128, 1], fp32, tag="wseed", name="wseed")

            def warm_chain(n, seed_ap=None):
                if seed_ap is not None:
                    # ACT scale APs must be fp32: bounce the (possibly bf16)
                    # seed through a fp32 copy to create the data dependency
                    nc.scalar.activation(wseed[:], seed_ap, AF.Copy)
                    nc.scalar.mul(wact[:], wact[:], wseed[:])
                for _ in range(n):
                    wmp = pp.tile([16, 16], fp32, tag="ck", name="wmp")
                    nc.scalar.mul(wact[:], wact[:], 1.0)
                    nc.tensor.matmul(wmp[:], k8[:, 0, 0:16], wact[:], start=True, stop=True)

            nc.scalar.activation(scr[:, 0:1], scr[:, 0:1], AF.Exp)

            wc = mp.tile([128, NCH, F], bf16, tag="wc", name="wc")
            s_sb = mp.tile([128, BT, F], bf16, tag="s_sb", name="s_sb")
            s_sb32 = mp.tile([128, BT, F], fp32, tag="s_sb32", name="s_sb32")
            v_bf = mp.tile([128, BT, F], bf16, tag="v_bf", name="v_bf")

            def squash(P, NH, s_ap, v_out_ap, scale, sfx):
                """v = squash(s) over u; s_ap/v_out_ap are [P, NH, F] APs.
                All-vector: fused int magic rsqrt seed + one Newton step."""
                sq = mp.tile([P, NH, F], fp32, tag=f"sq{sfx}", name=f"sq{sfx}")
                nc.vector.scalar_tensor_tensor(
                    out=sq[:], in0=s_ap, scalar=scale * scale, in1=s_ap,
                    op0=ALU.mult, op1=ALU.mult,
                )
                sqn = mp.tile([P, NH, OC], fp32, tag=f"sqn{sfx}", name=f"sqn{sfx}")
                nc.vector.reduce_sum(
                    sqn[:], sq[:].rearrange("p h (o u) -> p h o u", o=OC), axis=AX.X
                )
                # y = rsqrt seed: bits(y) = MAGIC - (bits(sqn) >> 1), all-int:
                #   t = ~(bits >> 1);  y = t + (MAGIC + 1)
                ti = mp.tile([P, NH, OC], i32, tag=f"ti{sfx}", name=f"ti{sfx}")
                nc.vector.tensor_scalar(
                    out=ti[:], in0=sqn[:].bitcast(i32), scalar1=1, scalar2=-1,
                    op0=ALU.logical_shift_right, op1=ALU.bitwise_xor,
                )
                yi = mp.tile([P, NH, OC], i32, tag=f"yi{sfx}", name=f"yi{sfx}")
                nc.vector.tensor_scalar(
                    out=yi[:], in0=ti[:], scalar1=MAGIC_I + 1, scalar2=None,
                    op0=ALU.add,
                )
                y = yi[:].bitcast(fp32)
                # Newton: y' = (1.5 - 0.5*sqn*y^2) * y
                t1 = mp.tile([P, NH, OC], fp32, tag=f"t1{sfx}", name=f"t1{sfx}")
                t2 = mp.tile([P, NH, OC], fp32, tag=f"t2{sfx}", name=f"t2{sfx}")
                y2 = mp.tile([P, NH, OC], fp32, tag=f"y2{sfx}", name=f"y2{sfx}")
                nc.vector.tensor_tensor(out=t1[:], in0=y, in1=y, op=ALU.mult)
                nc.vector.scalar_tensor_tensor(
                    out=t2[:], in0=t1[:], scalar=-0.5, in1=sqn[:],
                    op0=ALU.mult, op1=ALU.mult,
                )
                nc.vector.scalar_tensor_tensor(
                    out=y2[:], in0=t2[:], scalar=1.5, in1=y,
                    op0=ALU.add, op1=ALU.mult,
                )
                # f = scale * sqn * y / (1 + sqn)
                d1 = mp.tile([P, NH, OC], fp32, tag=f"d1{sfx}", name=f"d1{sfx}")
                nc.vector.tensor_scalar_add(d1[:], sqn[:], 1.0)
                dr = mp.tile([P, NH, OC], fp32, tag=f"dr{sfx}", name=f"dr{sfx}")
                nc.vector.reciprocal(dr[:], d1[:])
                f2 = mp.tile([P, NH, OC], fp32, tag=f"f2{sfx}", name=f"f2{sfx}")
                nc.vector.tensor_tensor(out=f2[:], in0=sqn[:], in1=y2[:], op=ALU.mult)
                ff = mp.tile([P, NH, OC], fp32, tag=f"ff{sfx}", name=f"ff{sfx}")
                nc.vector.scalar_tensor_tensor(
                    out=ff[:], in0=f2[:], scalar=scale, in1=dr[:],
                    op0=ALU.mult, op1=ALU.mult,
                )
                nc.vector.tensor_tensor(
                    out=v_out_ap.rearrange("p h (o u) -> p h o u", o=OC),
                    in0=s_ap.rearrange("p h (o u) -> p h o u", o=OC),
                    in1=ff[:].unsqueeze(3).broadcast_to([P, NH, OC, OU]),
                    op=ALU.mult,
                )

            for it in range(3):
                # ---- c = softmax(b) over o; Wc = W_t * c[i(p), o(f)] ----
                if it > 0:
                    e_all = mp.tile([128, 2, OC], fp32, tag="e_all", name="e_all")
                    nc.scalar.activation(e_all[:], b_all[:], AF.Exp)
                    r_all = mp.tile([128, 2], fp32, tag="r_all", name="r_all")
                    nc.vector.reduce_sum(r_all[:], e_all[:], axis=AX.X)
                    ri = mp.tile([128, 2], fp32, tag="ri", name="ri")
                    nc.vector.reciprocal(ri[:], r_all[:])
                    c_all = mp.tile([128, 2, OC], bf16, tag=f"c_all{it}", name=f"c_all{it}")
                    nc.vector.tensor_tensor(
                        out=c_all[:],
                        in0=e_all[:],
                        in1=ri[:].unsqueeze(2).broadcast_to([128, 2, OC]),
                        op=ALU.mult,
                    )
                # ---- partial s: accumulate 9 (i,k)-chunks per b-half ----
                s_ps = [
                    ppb.tile([128, F], fp32, tag=f"sps{h}", name=f"sps{h}")
                    for h in range(BT)
                ]

                def s_mm(ch):
                    rhs = wc[:, ch, :] if it > 0 else wt[:, ch, :]
                    for h in range(BT):
                        nc.tensor.matmul(
                            s_ps[h][:],
                            inpT[:, ch, 128 * h : 128 * (h + 1)],
                            rhs,
                            start=(ch == 0),
                            stop=(ch == NCH - 1),
                        )

                if it > 0:
                    # chunk pairs share one PSUM bank; one double-width Wc TT
                    # per pair. One-pair lookahead keeps the in-order PE queue
                    # from stalling on the vector engine.
                    pairs = [(0, 1), (2, 3), (4, 5), (6, 7), (8,)]
                    cks = {}

                    def cexp(cp):
                        chs = pairs[cp]
                        ckp = pp.tile([128, 2, OC], fp32, tag="ck", name="ckp")
                        for ci, ch in enumerate(chs):
                            if ch < 8:
                                a, m = divmod(ch, 4)
                                rhs = c_all[64 * a : 64 * a + 64, 0, :]
                                lhsT = k8[64 * a : 64 * a + 64, m, :]
                            else:
                                rhs = c_all[0:16, 1, :]
                                lhsT = k8[0:16, 0, :]
                            nc.tensor.matmul(ckp[:, ci, :], lhsT, rhs, start=True, stop=True)
                        cks[cp] = ckp

                    def wc_tt(cp):
                        chs = pairs[cp]
                        n = len(chs)
                        c0 = chs[0]
                        nc.vector.tensor_tensor(
                            out=wc[:, c0 : c0 + n, :].rearrange(
                                "p c (o u) -> p c o u", o=OC
                            ),
                            in0=wt[:, c0 : c0 + n, :].rearrange(
                                "p c (o u) -> p c o u", o=OC
                            ),
                            in1=cks[cp][:, 0:n, :]
                            .unsqueeze(3)
                            .broadcast_to([128, n, OC, OU]),
                            op=ALU.mult,
                        )

                    cexp(0)
                    for cp in range(len(pairs)):
                        if cp + 1 < len(pairs):
                            cexp(cp + 1)
                        wc_tt(cp)
                        for ch in pairs[cp]:
                            s_mm(ch)
                else:
                    for ch in range(NCH):
                        s_mm(ch)

                # ---- cross-core reduction of partial s ----
                if it < 2:
                    # bf16 payload: halves wire + bounce DMA time
                    s_bounce = dp.tile([B, F], bf16, tag="sb_in", name="sb_in")
                    for h in range(BT):
                        nc.scalar.copy(s_sb[:, h, :], s_ps[h][:])
                    nc.scalar.dma_start(
                        s_bounce.rearrange("(h p) f -> p h f", h=BT), s_sb[:]
                    )
                    s_red = s_red_sh[it][:]
                    nc.gpsimd.collective_compute(
                        "AllReduce",
                        ALU.add,
                        replica_groups=[list(range(N_CORES))],
                        ins=[s_bounce.opt()],
                        outs=[s_red.opt()],
                    )
                    warm_chain(14 if it == 1 else 12, seed_ap=s_sb[:, 0, 0:1])
                    nc.scalar.dma_start(
                        s_sb[:], s_red.rearrange("(h p) f -> p h f", h=BT)
                    )
                    scale = 0.1 if it == 0 else 1.0
                    # v straight to bf16 (only consumed by the next matmul)
                    squash(128, 2, s_sb[:], v_bf[:], scale, "a")
                else:
                    # final iteration: fp32 ReduceScatter + local squash of this
                    # core's 32-row slice; host concatenates the slices.
                    s_bounce32 = dp.tile([B, F], fp32, tag="sb32_in", name="sb32_in")
                    for h in range(BT):
                        nc.scalar.copy(s_sb32[:, h, :], s_ps[h][:])
                    nc.scalar.dma_start(
                        s_bounce32.rearrange("(h p) f -> p h f", h=BT), s_sb32[:]
                    )
                    rs_out = dp.tile([BSL, F], fp32, tag="rs_out", name="rs_out")
                    nc.gpsimd.collective_compute(
                        "ReduceScatter",
                        ALU.add,
                        replica_groups=[list(range(N_CORES))],
                        ins=[s_bounce32.opt()],
                        outs=[rs_out.opt()],
                    )
                    warm_chain(8, seed_ap=s_sb32[:, 0, 0:1])
                    s32 = mp.tile([BSL, 1, F], fp32, tag="s32", name="s32")
                    nc.scalar.dma_start(s32[:, 0, :], rs_out[:])
                    v32 = mp.tile([BSL, 1, F], fp32, tag="v32", name="v32")
                    squash(BSL, 1, s32[:], v32[:], 1.0, "b")
                    nc.scalar.dma_start(v_d[:], v32[:, 0, :])
                    continue

                # ---- b += (1/B) sum_{u,k} W_t * (inp^T @ v) ----
                dacc0 = ppb.tile([128, F], fp32, tag="dacc0", name="dacc0")
                dacc1 = ppb.tile([16, F], fp32, tag="dacc1", name="dacc1")
                mpairs = [(0, 1), (2, 3), (4, 5), (6, 7), (8,)]
                m2s, gs = {}, {}

                def m2_mm(cp):
                    chs = mpairs[cp]
                    # one accumulation group for the whole bank: start=True only
                    # on the first matmul (start clears the entire PSUM bank);
                    # has_written bits make the second half's first write an
                    # overwrite, not an accumulate.
                    m2 = pp.tile([128, 2, F], fp32, tag="m2", name="m2p")
                    nmm = len(chs) * BT
                    k = 0
                    for ci, ch in enumerate(chs):
                        for h in range(BT):
                            nc.tensor.matmul(
                                m2[:, ci, :],
                                inp_bk[:, h, 128 * ch : 128 * (ch + 1)],
                                v_bf[:, h, :],
                                start=(k == 0),
                                stop=(k == nmm - 1),
                                skip_group_check=True,
                            )
                            k += 1
                    m2s[cp] = m2

                def g_tt(cp):
                    chs = mpairs[cp]
                    n = len(chs)
                    g = mp.tile([128, 2, F], bf16, tag="g", name="gp")
                    nc.vector.tensor_tensor(
                        out=g[:, 0:n, :],
                        in0=wt[:, chs[0] : chs[0] + n, :],
                        in1=m2s[cp][:, 0:n, :],
                        op=ALU.mult,
                    )
                    gs[cp] = g

                def delta_mm(cp):
                    for ci, ch in enumerate(mpairs[cp]):
                        if ch < 8:
                            nc.tensor.matmul(
                                dacc0[:],
                                sw[:, 112 - 16 * ch : 240 - 16 * ch],
                                gs[cp][:, ci, :],
                                start=(ch == 0),
                                stop=(ch == 7),
                            )
                        else:
                            nc.tensor.matmul(
                                dacc1[:], sw[:, 112:128], gs[cp][:, ci, :],
                                start=True, stop=True,
                            )

                m2_mm(0)
                for cp in range(len(mpairs)):
                    if cp + 1 < len(mpairs):
                        m2_mm(cp + 1)
                    g_tt(cp)
                    delta_mm(cp)
                x0 = mp.tile([128, OC], fp32, tag="x0", name="x0")
                x1 = mp.tile([16, OC], fp32, tag="x1", name="x1")
                nc.vector.reduce_sum(
                    x0[:], dacc0[:].rearrange("p (o u) -> p o u", o=OC), axis=AX.X
                )
                nc.vector.reduce_sum(
                    x1[:], dacc1[:].rearrange("p (o u) -> p o u", o=OC), axis=AX.X
                )
                nc.vector.tensor_add(b_all[:, 0, :], b_all[:, 0, :], x0[:])
                nc.vector.tensor_add(b_all[0:16, 1, :], b_all[0:16, 1, :], x1[:])

    nc.compile()
    return nc


def _prep_inputs(inp, W):
    import ml_dtypes

    bf = ml_dtypes.bfloat16
    inp = np.ascontiguousarray(inp, dtype=np.float32)
    W = np.ascontiguousarray(W, dtype=np.float32)

    k8 = np.zeros((128, 4, 128), dtype=bf)
    q = np.arange(128)
    p = np.arange(128)
    for m in range(4):
        k8[:, m, :] = ((q % 64)[:, None] == (16 * m + p // 8)[None, :]).astype(bf)
    sw = np.zeros((128, 240), dtype=np.float32)
    j = np.arange(240)
    sw[:, :] = (j[None, :] == (112 + p // 8)[:, None]).astype(np.float32) / float(B)
    sw = sw.astype(bf)

    in_maps = []
    for jc in range(N_CORES):
        inp_bk = inp[:, jc::N_CORES, :].reshape(B, PK)
        inp_bk_d = np.ascontiguousarray(
            inp_bk.reshape(BT, 128, PK).transpose(1, 0, 2)
        ).astype(bf)
        inpT_d = np.ascontiguousarray(
            inp_bk.T.reshape(NCH, 128, B).transpose(1, 0, 2)
        ).astype(bf)
        wt = W[jc::N_CORES].transpose(0, 3, 1, 2).reshape(PK, F)
        wt_d = np.ascontiguousarray(wt.reshape(NCH, 128, F).transpose(1, 0, 2)).astype(bf)
        in_maps.append(
            {
                "inp_bk": inp_bk_d,
                "inpT": inpT_d,
                "wt": wt_d,
                "k8": k8,
                "sw": sw,
            }
        )
    return in_maps


def kernel(inp, W):
    from concourse.bass_utils import run_bass_kernel_spmd

    if "nc" not in _CACHE:
        _CACHE["nc"] = _build()
    nc = _CACHE["nc"]
    in_maps = _prep_inputs(inp, W)
    res = run_bass_kernel_spmd(nc, in_maps, list(range(N_CORES)))
    v = np.concatenate([res.results[j]["v_out"] for j in range(N_CORES)], axis=0)
    return np.ascontiguousarray(v.reshape(B, OC, OU)).astype(np.float32)
